# revision 42
# baseline (speedup 1.0000x reference)
"""Trainium2 Bass kernel for nn_AttentionBlock (B=2, S=2048, HID=2048, 16 heads, HD=128).

Sharding: 8 cores = 2 (batch) x 4 (head-groups of 4 heads). Each core computes
q/k/v projections for its heads, RoPE + per-head QK RMSNorm, causal attention,
and a partial w_o projection; the host sums the 4 partial outputs per batch.

All matmuls run as float32r (single-pass, ~fp22 operand precision, fp32
accumulate). Activations stay fp32 on-chip. x is streamed in s-column chunks
so q/k stay SBUF-resident end-to-end (no DRAM spill).
"""
import sys
import math
from contextlib import ExitStack

sys.path.insert(0, "/opt/trn_rl_repo")

import numpy as np

import concourse.bass as bass
import concourse.tile as tile
from concourse import bacc, mybir
from concourse.bass_utils import run_bass_kernel_spmd

f32 = mybir.dt.float32
f32r = mybir.dt.float32r
AF = mybir.ActivationFunctionType
ALU = mybir.AluOpType

B, S_FULL, HID = 2, 2048, 2048
NH, HD = 16, 128
HPC = 4                      # heads per core
NCORES = 8
ROPE_BASE = 10000.0
EPS = 1e-6
SCALE = 1.0 / math.sqrt(HD)
NEG = -1e30
NROPE = 3                    # rope pairs actually rotated (j>=3: |sin| < 2.1e-9)
NHT = 2 * HPC                # 8 head-tensors per core (4 q + 4 k)
NR = NHT * NROPE             # packed rope rows


def build_kernel(S=S_FULL, KD=HID, MO=HID, R=1):
    KB = KD // 128
    SC_N = S // 512
    SB_N = S // 128
    MO_N = MO // 128

    nc = bacc.Bacc("TRN2", target_bir_lowering=False, debug=False,
                   num_devices=NCORES)
    XTC = nc.dram_tensor("xtc", [SC_N, 128, KB, 512], f32r,
                         kind="ExternalInput").ap()
    WQK = nc.dram_tensor("wqk", [NHT, 128, KB, 128], f32r,
                         kind="ExternalInput").ap()
    WV = nc.dram_tensor("wv", [128, KB, HPC * 128], f32r,
                        kind="ExternalInput").ap()
    WOT = nc.dram_tensor("wot", [MO // 128, 128, HPC, 128], f32r,
                         kind="ExternalInput").ap()
    CC = nc.dram_tensor("cc", [NR, S], f32, kind="ExternalInput").ap()
    SS = nc.dram_tensor("ss", [NR, S], f32, kind="ExternalInput").ap()
    GSC = nc.dram_tensor("gsc", [128], f32, kind="ExternalInput").ap()
    MB = nc.dram_tensor("mb", [128, SB_N], f32, kind="ExternalInput").ap()
    TRI = nc.dram_tensor("tri", [128, 128], f32r, kind="ExternalInput").ap()
    ONE = nc.dram_tensor("one", [128], f32r, kind="ExternalInput").ap()
    PO = nc.dram_tensor("po", [MO, S], f32, kind="ExternalOutput").ap()
    # internal DRAM scratch (invq partition-broadcast bounce)
    IQD = nc.dram_tensor("invq_d", [HPC, 128, SB_N], f32).ap()

    with tile.TileContext(nc) as tc, ExitStack() as gctx:
        singles = gctx.enter_context(tc.tile_pool(name="singles", bufs=1))
        gsc_sb = singles.tile([128, 1], f32)
        nc.sync.dma_start(gsc_sb[:], GSC[:, None])
        mb_sb = singles.tile([128, SB_N], f32)
        nc.gpsimd.dma_start(mb_sb[:], MB)
        tri_sb = singles.tile([128, 128], f32r)
        nc.gpsimd.dma_start(tri_sb[:], TRI)
        ones_sb = singles.tile([128, 1], f32r)
        nc.gpsimd.dma_start(ones_sb[:], ONE[:, None])
        ones_row = singles.tile([1, 128], f32r)
        nc.gpsimd.dma_start(ones_row[:], ONE[None, :])
        ones2_sb = singles.tile([128, 2], f32r)
        nc.sync.dma_start(ones2_sb[:, 0:1], ONE[:, None])
        nc.sync.dma_start(ones2_sb[:, 1:2], ONE[:, None])
        eps_sb = singles.tile([128, 1], f32)
        nc.vector.memset(eps_sb[:], EPS)

        for _rep in range(R):
            with ExitStack() as rctx:
                rs = rctx.enter_context(tc.tile_pool(name="res", bufs=1))
                inv_all = rs.tile([128, NHT, SB_N], f32, name="inv_all")
                srt = rs.tile([128, NHT, SB_N], f32, name="srt")
                v_sb = rs.tile([128, SB_N, HPC * 128], f32r, name="v_sb")
                qks = [rs.tile([128, S], f32r, name=f"qk{t}")
                       for t in range(NHT)]
                # ---- P1: projections, x streamed in s-column chunks ----
                with ExitStack() as p1:
                    wv_pool = p1.enter_context(tc.tile_pool(name="wvp",
                                                            bufs=1))
                    wv_sb = wv_pool.tile([128, KB, HPC * 128], f32r,
                                         name="wv_sb")
                    for kb in range(KB):
                        nc.gpsimd.dma_start(wv_sb[:, kb, :], WV[:, kb, :])
                    xt_pool = p1.enter_context(tc.tile_pool(name="xtc", bufs=5))
                    w_pool = p1.enter_context(tc.tile_pool(name="wqk", bufs=2))
                    sq_pool = p1.enter_context(tc.tile_pool(name="sq", bufs=2))
                    rp_pool = p1.enter_context(tc.tile_pool(name="rope",
                                                             bufs=1))
                    ps_pool = p1.enter_context(
                        tc.tile_pool(name="ps1", bufs=5, space="PSUM"))
                    ssq_pool = p1.enter_context(
                        tc.tile_pool(name="ssq", bufs=1, space="PSUM"))
                    ssq_ps = ssq_pool.tile([128, NHT, SB_N, 2], f32)
                    KQ = KB // 4
                    for c in range(SC_N):
                        xtq = []
                        for qq in range(4):
                            xq = xt_pool.tile([128, KQ, 512], f32r, tag="xtc",
                                              name=f"xt{qq}")
                            nc.sync.dma_start(
                                xq[:], XTC[c, :, qq * KQ:(qq + 1) * KQ, :])
                            xtq.append(xq)

                        def xts(kb):
                            return xtq[kb // KQ][:, kb % KQ, :]
                        for ht in range(NHT):
                            w_sb = w_pool.tile([128, KB, 128], f32r, tag="w")
                            if c == 0 and ht == 0:
                                for qq in range(4):
                                    nc.scalar.dma_start(
                                        w_sb[:, qq * KQ:(qq + 1) * KQ, :],
                                        WQK[ht, :, qq * KQ:(qq + 1) * KQ, :])
                            else:
                                nc.scalar.dma_start(w_sb[:], WQK[ht])
                            psq = ps_pool.tile([128, 512], f32, tag="ps1t")
                            for kb in range(KB):
                                nc.tensor.matmul(
                                    psq[:], w_sb[:, kb, :], xts(kb),
                                    start=(kb == 0), stop=(kb == KB - 1))
                            nc.vector.tensor_copy(
                                qks[ht][:, c * 512:(c + 1) * 512], psq[:])
                            sq = sq_pool.tile([128, 512], f32r, tag="sqt")
                            nc.scalar.activation(sq[:], psq[:], AF.Square)
                            for s2 in range(4):
                                blk = c * 4 + s2
                                nc.tensor.matmul(
                                    ssq_ps[:, ht, blk, :],
                                    sq[:, s2 * 128:(s2 + 1) * 128],
                                    ones2_sb[:], start=True, stop=True,
                                    skip_group_check=True)
                        if c == SC_N - 1:
                            # invrms = 1/sqrt(ssq/HD + eps); emitted before
                            # rope so recip isn't queued behind it on DVE
                            nc.scalar.activation(
                                srt[:].rearrange("p a b -> p (a b)"),
                                ssq_ps[:, :, :, 0].rearrange(
                                    "p a b -> p (a b)"),
                                AF.Sqrt, bias=eps_sb[:], scale=1.0 / HD)
                            nc.vector.reciprocal(
                                inv_all[:].rearrange("p a b -> p (a b)"),
                                srt[:].rearrange("p a b -> p (a b)"))
                            for h in range(HPC):
                                nc.sync.dma_start(IQD[h], inv_all[:, h, :])
                        # per-chunk packed rope on the 6 active rows/ht
                        cs = slice(c * 512, (c + 1) * 512)
                        X1 = rp_pool.tile([NR, 512], f32, tag="X1")
                        X2 = rp_pool.tile([NR, 512], f32, tag="X2")
                        for ht in range(NHT):
                            nc.scalar.dma_start(
                                X1[ht * NROPE:(ht + 1) * NROPE, :],
                                qks[ht][0:NROPE, cs].bitcast(f32))
                            nc.scalar.dma_start(
                                X2[ht * NROPE:(ht + 1) * NROPE, :],
                                qks[ht][64:64 + NROPE, cs].bitcast(f32))
                        cc_sb = rp_pool.tile([NR, 512], f32, tag="ccs")
                        ss_sb = rp_pool.tile([NR, 512], f32, tag="sss")
                        nc.sync.dma_start(cc_sb[:], CC[:, cs])
                        nc.sync.dma_start(ss_sb[:], SS[:, cs])
                        tA = rp_pool.tile([NR, 512], f32, tag="tA")
                        tB = rp_pool.tile([NR, 512], f32, tag="tB")
                        nc.vector.tensor_tensor(tA[:], X1[:], cc_sb[:],
                                                ALU.mult)
                        nc.vector.tensor_tensor(X1[:], X1[:], ss_sb[:],
                                                ALU.mult)
                        nc.vector.tensor_tensor(tB[:], X2[:], cc_sb[:],
                                                ALU.mult)
                        nc.vector.tensor_tensor(X2[:], X2[:], ss_sb[:],
                                                ALU.mult)
                        nc.vector.tensor_tensor(tA[:], tA[:], X2[:],
                                                ALU.subtract)
                        nc.vector.tensor_tensor(tB[:], tB[:], X1[:], ALU.add)
                        for ht in (0, 4, 1, 5, 2, 6, 3, 7):
                            nc.gpsimd.dma_start(
                                qks[ht][0:NROPE, cs],
                                tA[ht * NROPE:(ht + 1) * NROPE, :]
                                .bitcast(f32r))
                            nc.gpsimd.dma_start(
                                qks[ht][64:64 + NROPE, cs],
                                tB[ht * NROPE:(ht + 1) * NROPE, :]
                                .bitcast(f32r))
                        # V projection for this chunk's 4 s-blocks
                        for s2 in range(4):
                            sb = c * 4 + s2
                            psv = ps_pool.tile([128, HPC * 128], f32,
                                               tag="ps1t")
                            for kb in range(KB):
                                nc.tensor.matmul(
                                    psv[:],
                                    xts(kb)[:, s2 * 128:(s2 + 1) * 128],
                                    wv_sb[:, kb, :],
                                    start=(kb == 0), stop=(kb == KB - 1))
                            nc.scalar.copy(v_sb[:, sb, :], psv[:])

                # ---- P2/P3: attention + output proj, chunk-outer ----
                with ExitStack() as p23:
                    at_pool = p23.enter_context(tc.tile_pool(name="attn",
                                                             bufs=1))
                    attn_sb = at_pool.tile([128, HPC, S], f32r)
                    wo_pool = p23.enter_context(tc.tile_pool(name="wo",
                                                             bufs=1))
                    pt_pool = p23.enter_context(tc.tile_pool(name="pt",
                                                             bufs=4))
                    zs_pool = p23.enter_context(tc.tile_pool(name="zs",
                                                             bufs=2))
                    po_pool = p23.enter_context(tc.tile_pool(name="po",
                                                             bufs=3))
                    ps_s = p23.enter_context(tc.tile_pool(name="ps_s", bufs=2,
                                                          space="PSUM"))
                    ps_z = p23.enter_context(tc.tile_pool(name="ps_z", bufs=1,
                                                          space="PSUM"))
                    ps_bc = p23.enter_context(tc.tile_pool(name="ps_bc",
                                                           bufs=1,
                                                           space="PSUM"))
                    ps_o = p23.enter_context(tc.tile_pool(name="ps_o", bufs=2,
                                                          space="PSUM"))
                    ps_po = p23.enter_context(tc.tile_pool(name="ps_po",
                                                           bufs=2,
                                                           space="PSUM"))
                    bc_pool2 = p23.enter_context(tc.tile_pool(name="bc2",
                                                              bufs=2))
                    for h in range(HPC):
                        q_sb = qks[h]
                        bcq = bc_pool2.tile([128, 128, SB_N], f32, tag="bcq2")
                        nc.sync.dma_start(
                            bcq[:].rearrange("p a b -> p (a b)"),
                            IQD[h].rearrange("p b -> (p b)")[None, :]
                            .to_broadcast((128, 128 * SB_N)))
                        qv = q_sb[:].rearrange("p (blk sp) -> p sp blk",
                                               sp=128)
                        nc.vector.scalar_tensor_tensor(
                            qv, qv, gsc_sb[:], bcq[:], ALU.mult, ALU.mult)
                    wos = []
                    for mo in range(MO_N):
                        wo_sb = wo_pool.tile([128, HPC, 128], f32r,
                                             tag=f"wo{mo}", name=f"wo{mo}")
                        nc.scalar.dma_start(wo_sb[:], WOT[mo])
                        wos.append(wo_sb)
                    for c in range(SC_N):
                        for h in range(HPC):
                            q_sb, k_sb = qks[h], qks[HPC + h]
                            oc = ps_o.tile([128, 512], f32, tag="o")
                            zc = ps_z.tile([1, 512], f32, tag="z")
                            nblk = 4 * c + 4
                            for i in range(nblk):
                                r = i - 4 * c
                                off = 128 * r if r > 0 else 0
                                sps = ps_s.tile([128, 512], f32, tag="s")
                                nc.tensor.matmul(
                                    sps[:, off:512],
                                    k_sb[:, i * 128:(i + 1) * 128],
                                    q_sb[:, c * 512 + off:(c + 1) * 512],
                                    start=True, stop=True,
                                    skip_group_check=True)
                                pt = pt_pool.tile([128, 512], f32r, tag="pt")
                                nc.scalar.activation(
                                    pt[:, off:512], sps[:, off:512], AF.Exp,
                                    bias=mb_sb[:, i:i + 1],
                                    scale=inv_all[:, HPC + h, i:i + 1])
                                if r >= 0:
                                    nc.vector.tensor_tensor(
                                        pt[:, off:off + 128],
                                        pt[:, off:off + 128],
                                        tri_sb[:], ALU.mult)
                                nc.tensor.matmul(
                                    zc[:, off:512], ones_sb[:], pt[:, off:512],
                                    start=(i == 0), stop=(i == nblk - 1),
                                    skip_group_check=True)
                                nc.tensor.matmul(
                                    oc[:, off:512],
                                    v_sb[:, i, h * 128:(h + 1) * 128],
                                    pt[:, off:512],
                                    start=(i == 0), stop=(i == nblk - 1),
                                    skip_group_check=True)
                            nc.scalar.copy(
                                attn_sb[:, h, c * 512:(c + 1) * 512], oc[:])
                            # invZ via reciprocal + K=1 broadcast matmul
                            izrow = zs_pool.tile([1, 512], f32r, tag="izrow")
                            with nc.allow_low_precision(
                                    reason="invZ broadcast via f32r matmul"):
                                nc.vector.reciprocal(izrow[:], zc[:])
                            bcz_ps = ps_bc.tile([128, 512], f32, tag="bcz")
                            nc.tensor.matmul(bcz_ps[:], ones_row[:],
                                             izrow[:], start=True, stop=True,
                                             skip_group_check=True)
                            nc.vector.tensor_tensor(
                                attn_sb[:, h, c * 512:(c + 1) * 512],
                                attn_sb[:, h, c * 512:(c + 1) * 512],
                                bcz_ps[:], ALU.mult)
                        # output projection for this chunk
                        for mo in range(MO_N):
                            pp = ps_po.tile([128, 512], f32, tag="pp")
                            for jb in range(HPC):
                                nc.tensor.matmul(
                                    pp[:], wos[mo][:, jb, :],
                                    attn_sb[:, jb, c * 512:(c + 1) * 512],
                                    start=(jb == 0), stop=(jb == HPC - 1))
                            ob = po_pool.tile([128, 512], f32, tag="ob")
                            if mo % 2 == 0:
                                nc.scalar.copy(ob[:], pp[:])
                            else:
                                nc.vector.tensor_copy(ob[:], pp[:])
                            nc.gpsimd.dma_start(
                                PO[mo * 128:(mo + 1) * 128,
                                   c * 512:(c + 1) * 512], ob[:])
    nc.compile()
    return nc


def rope_tables(S):
    j = np.arange(0, HD, 2, dtype=np.float32)
    with np.errstate(over="ignore", divide="ignore"):
        freq = np.exp(np.float32(math.log(ROPE_BASE)) * j)
        inv_freq = (np.float32(1.0) / freq).astype(np.float32)
    t = np.arange(S, dtype=np.float32)
    freqs = t[:, None] * inv_freq[None, :]
    return np.cos(freqs).astype(np.float32), np.sin(freqs).astype(np.float32)


def pack_core(x2d, wq4, wk4, wv4, wo_sh, qnw, knw, mask_row, S, KD, MO):
    """Per-core input map. x2d: [S, KD]; wq4/wk4/wv4: [512, KD] weight rows for
    this core's heads; wo_sh: [MO, 512] = w_o columns for these heads."""
    SB_N = S // 128
    SC_N = S // 512
    MO_N = MO // 128
    KB = KD // 128
    f = np.float32
    xtc = np.ascontiguousarray(
        x2d.T.reshape(KB, 128, SC_N, 512).transpose(2, 1, 0, 3), dtype=f)
    wqk = np.empty((NHT, 128, KB, 128), f)
    for hh in range(HPC):
        wqk[hh] = wq4[hh * 128:(hh + 1) * 128, :].T.reshape(
            KB, 128, 128).transpose(1, 0, 2)
        wqk[HPC + hh] = wk4[hh * 128:(hh + 1) * 128, :].T.reshape(
            KB, 128, 128).transpose(1, 0, 2)
    wv = np.ascontiguousarray(
        wv4.T.reshape(KB, 128, HPC * 128).transpose(1, 0, 2), dtype=f)
    wot = np.ascontiguousarray(
        wo_sh.reshape(MO_N, 128, HPC, 128).transpose(0, 3, 2, 1), dtype=f)
    cos, sin = rope_tables(S)
    cc = np.empty((NR, S), f)
    ss = np.empty((NR, S), f)
    for ht in range(NHT):
        for jj in range(NROPE):
            cc[ht * NROPE + jj] = cos[:, jj]
            ss[ht * NROPE + jj] = sin[:, jj]
    gsc = (qnw * knw * np.float32(SCALE)).astype(f)
    mb = np.where(mask_row.reshape(SB_N, 128).T, f(0.0), f(NEG)).astype(f)
    tri = (np.arange(128)[None, :] >= np.arange(128)[:, None]).astype(f)
    return {
        "xtc": xtc, "wqk": wqk, "wv": wv, "wot": wot, "cc": cc, "ss": ss,
        "gsc": gsc, "mb": mb, "tri": tri, "one": np.ones(128, f),
    }


_cache = {}


def make_in_maps(inputs):
    x = np.asarray(inputs["x"], np.float32)
    w_q = np.asarray(inputs["w_q"], np.float32)
    w_k = np.asarray(inputs["w_k"], np.float32)
    w_v = np.asarray(inputs["w_v"], np.float32)
    w_o = np.asarray(inputs["w_o"], np.float32)
    qnw = np.asarray(inputs["q_norm_w"], np.float32)
    knw = np.asarray(inputs["k_norm_w"], np.float32)
    qmask = np.asarray(inputs["query_mask"]).astype(bool)
    in_maps = []
    for core in range(NCORES):
        b, hg = core // 4, core % 4
        rows = slice(hg * 512, (hg + 1) * 512)
        in_maps.append(pack_core(
            x[b], w_q[rows], w_k[rows], w_v[rows], w_o[:, rows],
            qnw, knw, qmask[b], S_FULL, HID, HID))
    return in_maps


def run_traced(inputs):
    if "nc" not in _cache:
        _cache["nc"] = build_kernel()
    return run_bass_kernel_spmd(_cache["nc"], make_in_maps(inputs),
                                core_ids=list(range(NCORES)), trace=True)


def kernel(**inputs):
    qmask = np.asarray(inputs["query_mask"]).astype(bool)
    if "nc" not in _cache:
        _cache["nc"] = build_kernel()
    nc = _cache["nc"]
    in_maps = make_in_maps(inputs)
    res = run_bass_kernel_spmd(nc, in_maps, core_ids=list(range(NCORES)))
    out = np.zeros((B, S_FULL, HID), np.float32)
    for core in range(NCORES):
        b = core // 4
        out[b] += res.results[core]["po"].T
    out = np.where(qmask[:, :, None], out, np.float32(0.0))
    return out.astype(np.float32)


# revision 43
# speedup vs baseline: 1.0382x; 1.0382x over previous
"""Trainium2 Bass kernel for nn_AttentionBlock (B=2, S=2048, HID=2048, 16 heads, HD=128).

Sharding: 8 cores = 2 (batch) x 4 (head-groups of 4 heads). Each core computes
q/k/v projections for its heads, RoPE + per-head QK RMSNorm, causal attention,
and a partial w_o projection; the host sums the 4 partial outputs per batch.

All matmuls run as float32r (single-pass, ~fp22 operand precision, fp32
accumulate). Activations stay fp32 on-chip. x is streamed in s-column chunks
so q/k stay SBUF-resident end-to-end (no DRAM spill).
"""
import sys
import math
from contextlib import ExitStack

sys.path.insert(0, "/opt/trn_rl_repo")

import numpy as np

import concourse.bass as bass
import concourse.tile as tile
from concourse import bacc, mybir
from concourse.bass_utils import run_bass_kernel_spmd

f32 = mybir.dt.float32
f32r = mybir.dt.float32r
AF = mybir.ActivationFunctionType
ALU = mybir.AluOpType

B, S_FULL, HID = 2, 2048, 2048
NH, HD = 16, 128
HPC = 4                      # heads per core
NCORES = 8
ROPE_BASE = 10000.0
EPS = 1e-6
SCALE = 1.0 / math.sqrt(HD)
NEG = -1e30
NROPE = 3                    # rope pairs actually rotated (j>=3: |sin| < 2.1e-9)
NHT = 2 * HPC                # 8 head-tensors per core (4 q + 4 k)
NR = NHT * NROPE             # packed rope rows


def build_kernel(S=S_FULL, KD=HID, MO=HID, R=1):
    KB = KD // 128
    SC_N = S // 512
    SB_N = S // 128
    MO_N = MO // 128

    nc = bacc.Bacc("TRN2", target_bir_lowering=False, debug=False,
                   num_devices=NCORES)
    XTC = nc.dram_tensor("xtc", [SC_N, 128, KB, 512], f32r,
                         kind="ExternalInput").ap()
    WQK = nc.dram_tensor("wqk", [NHT, 128, KB, 128], f32r,
                         kind="ExternalInput").ap()
    WV = nc.dram_tensor("wv", [128, KB, HPC * 128], f32r,
                        kind="ExternalInput").ap()
    WOT = nc.dram_tensor("wot", [MO // 128, 128, HPC, 128], f32r,
                         kind="ExternalInput").ap()
    CC = nc.dram_tensor("cc", [NR, S], f32, kind="ExternalInput").ap()
    SS = nc.dram_tensor("ss", [NR, S], f32, kind="ExternalInput").ap()
    GSC = nc.dram_tensor("gsc", [128], f32, kind="ExternalInput").ap()
    MB = nc.dram_tensor("mb", [128, SB_N], f32, kind="ExternalInput").ap()
    TRI = nc.dram_tensor("tri", [128, 128], f32r, kind="ExternalInput").ap()
    ONE = nc.dram_tensor("one", [128], f32r, kind="ExternalInput").ap()
    PO = nc.dram_tensor("po", [MO, S], f32, kind="ExternalOutput").ap()
    # internal DRAM scratch (invq partition-broadcast bounce)
    IQD = nc.dram_tensor("invq_d", [HPC, 128, SB_N], f32).ap()

    with tile.TileContext(nc) as tc, ExitStack() as gctx:
        singles = gctx.enter_context(tc.tile_pool(name="singles", bufs=1))
        gsc_sb = singles.tile([128, 1], f32)
        nc.sync.dma_start(gsc_sb[:], GSC[:, None])
        mb_sb = singles.tile([128, SB_N], f32)
        nc.gpsimd.dma_start(mb_sb[:], MB)
        tri_sb = singles.tile([128, 128], f32r)
        nc.gpsimd.dma_start(tri_sb[:], TRI)
        ones_sb = singles.tile([128, 1], f32r)
        nc.gpsimd.dma_start(ones_sb[:], ONE[:, None])
        ones_row = singles.tile([1, 128], f32r)
        nc.gpsimd.dma_start(ones_row[:], ONE[None, :])
        ones2_sb = singles.tile([128, 2], f32r)
        nc.sync.dma_start(ones2_sb[:, 0:1], ONE[:, None])
        nc.sync.dma_start(ones2_sb[:, 1:2], ONE[:, None])
        eps_sb = singles.tile([128, 1], f32)
        nc.vector.memset(eps_sb[:], EPS)

        for _rep in range(R):
            with ExitStack() as rctx:
                rs = rctx.enter_context(tc.tile_pool(name="res", bufs=1))
                inv_all = rs.tile([128, NHT, SB_N], f32, name="inv_all")
                srt = rs.tile([128, NHT, SB_N], f32, name="srt")
                v_sb = rs.tile([128, SB_N, HPC * 128], f32r, name="v_sb")
                qks = [rs.tile([128, S], f32r, name=f"qk{t}")
                       for t in range(NHT)]
                # ---- P1: projections, x streamed in s-column chunks ----
                with ExitStack() as p1:
                    wv_pool = p1.enter_context(tc.tile_pool(name="wvp",
                                                            bufs=1))
                    wv_sb = wv_pool.tile([128, KB, HPC * 128], f32r,
                                         name="wv_sb")
                    for kb in range(0, KB, 2):
                        nc.gpsimd.dma_start(wv_sb[:, kb, :], WV[:, kb, :])
                    xt_pool = p1.enter_context(tc.tile_pool(name="xtc", bufs=5))
                    w_pool = p1.enter_context(tc.tile_pool(name="wqk", bufs=2))
                    sq_pool = p1.enter_context(tc.tile_pool(name="sq", bufs=2))
                    rp_pool = p1.enter_context(tc.tile_pool(name="rope",
                                                             bufs=1))
                    ps_pool = p1.enter_context(
                        tc.tile_pool(name="ps1", bufs=5, space="PSUM"))
                    ssq_pool = p1.enter_context(
                        tc.tile_pool(name="ssq", bufs=1, space="PSUM"))
                    ssq_ps = ssq_pool.tile([128, NHT, SB_N, 2], f32)
                    KQ = KB // 4
                    for c in range(SC_N):
                        xtq = []
                        for qq in range(4):
                            xq = xt_pool.tile([128, KQ, 512], f32r, tag="xtc",
                                              name=f"xt{qq}")
                            nc.sync.dma_start(
                                xq[:], XTC[c, :, qq * KQ:(qq + 1) * KQ, :])
                            xtq.append(xq)
                        if c == 0:
                            for kb in range(1, KB, 2):
                                nc.sync.dma_start(wv_sb[:, kb, :],
                                                  WV[:, kb, :])

                        def xts(kb):
                            return xtq[kb // KQ][:, kb % KQ, :]
                        for ht in range(NHT):
                            w_sb = w_pool.tile([128, KB, 128], f32r, tag="w")
                            if c == 0 and ht == 0:
                                for qq in range(4):
                                    nc.scalar.dma_start(
                                        w_sb[:, qq * KQ:(qq + 1) * KQ, :],
                                        WQK[ht, :, qq * KQ:(qq + 1) * KQ, :])
                            else:
                                nc.scalar.dma_start(w_sb[:], WQK[ht])
                            psq = ps_pool.tile([128, 512], f32, tag="ps1t")
                            for kb in range(KB):
                                nc.tensor.matmul(
                                    psq[:], w_sb[:, kb, :], xts(kb),
                                    start=(kb == 0), stop=(kb == KB - 1))
                            nc.vector.tensor_copy(
                                qks[ht][:, c * 512:(c + 1) * 512], psq[:])
                            sq = sq_pool.tile([128, 512], f32r, tag="sqt")
                            nc.scalar.activation(sq[:], psq[:], AF.Square)
                            for s2 in range(4):
                                blk = c * 4 + s2
                                nc.tensor.matmul(
                                    ssq_ps[:, ht, blk, :],
                                    sq[:, s2 * 128:(s2 + 1) * 128],
                                    ones2_sb[:], start=True, stop=True,
                                    skip_group_check=True)
                        if c == SC_N - 1:
                            # invrms = 1/sqrt(ssq/HD + eps); emitted before
                            # rope so recip isn't queued behind it on DVE
                            nc.scalar.activation(
                                srt[:].rearrange("p a b -> p (a b)"),
                                ssq_ps[:, :, :, 0].rearrange(
                                    "p a b -> p (a b)"),
                                AF.Sqrt, bias=eps_sb[:], scale=1.0 / HD)
                            nc.vector.reciprocal(
                                inv_all[:].rearrange("p a b -> p (a b)"),
                                srt[:].rearrange("p a b -> p (a b)"))
                            for h in range(HPC):
                                nc.sync.dma_start(IQD[h], inv_all[:, h, :])
                        # per-chunk packed rope on the 6 active rows/ht
                        cs = slice(c * 512, (c + 1) * 512)
                        X1 = rp_pool.tile([NR, 512], f32, tag="X1")
                        X2 = rp_pool.tile([NR, 512], f32, tag="X2")
                        for ht in range(NHT):
                            nc.scalar.dma_start(
                                X1[ht * NROPE:(ht + 1) * NROPE, :],
                                qks[ht][0:NROPE, cs].bitcast(f32))
                            nc.scalar.dma_start(
                                X2[ht * NROPE:(ht + 1) * NROPE, :],
                                qks[ht][64:64 + NROPE, cs].bitcast(f32))
                        cc_sb = rp_pool.tile([NR, 512], f32, tag="ccs")
                        ss_sb = rp_pool.tile([NR, 512], f32, tag="sss")
                        nc.sync.dma_start(cc_sb[:], CC[:, cs])
                        nc.sync.dma_start(ss_sb[:], SS[:, cs])
                        tA = rp_pool.tile([NR, 512], f32, tag="tA")
                        tB = rp_pool.tile([NR, 512], f32, tag="tB")
                        nc.vector.tensor_tensor(tA[:], X1[:], cc_sb[:],
                                                ALU.mult)
                        nc.vector.tensor_tensor(X1[:], X1[:], ss_sb[:],
                                                ALU.mult)
                        nc.vector.tensor_tensor(tB[:], X2[:], cc_sb[:],
                                                ALU.mult)
                        nc.vector.tensor_tensor(X2[:], X2[:], ss_sb[:],
                                                ALU.mult)
                        nc.vector.tensor_tensor(tA[:], tA[:], X2[:],
                                                ALU.subtract)
                        nc.vector.tensor_tensor(tB[:], tB[:], X1[:], ALU.add)
                        for ht in (0, 4, 1, 5, 2, 6, 3, 7):
                            nc.gpsimd.dma_start(
                                qks[ht][0:NROPE, cs],
                                tA[ht * NROPE:(ht + 1) * NROPE, :]
                                .bitcast(f32r))
                            nc.gpsimd.dma_start(
                                qks[ht][64:64 + NROPE, cs],
                                tB[ht * NROPE:(ht + 1) * NROPE, :]
                                .bitcast(f32r))
                        # V projection for this chunk's 4 s-blocks
                        for s2 in range(4):
                            sb = c * 4 + s2
                            psv = ps_pool.tile([128, HPC * 128], f32,
                                               tag="ps1t")
                            for kb in range(KB):
                                nc.tensor.matmul(
                                    psv[:],
                                    xts(kb)[:, s2 * 128:(s2 + 1) * 128],
                                    wv_sb[:, kb, :],
                                    start=(kb == 0), stop=(kb == KB - 1))
                            nc.scalar.copy(v_sb[:, sb, :], psv[:])

                # ---- P2/P3: attention + output proj, chunk-outer ----
                with ExitStack() as p23:
                    at_pool = p23.enter_context(tc.tile_pool(name="attn",
                                                             bufs=1))
                    attn_sb = at_pool.tile([128, HPC, S], f32r)
                    wo_pool = p23.enter_context(tc.tile_pool(name="wo",
                                                             bufs=1))
                    pt_pool = p23.enter_context(tc.tile_pool(name="pt",
                                                             bufs=4))
                    zs_pool = p23.enter_context(tc.tile_pool(name="zs",
                                                             bufs=2))
                    po_pool = p23.enter_context(tc.tile_pool(name="po",
                                                             bufs=3))
                    ps_s = p23.enter_context(tc.tile_pool(name="ps_s", bufs=2,
                                                          space="PSUM"))
                    ps_z = p23.enter_context(tc.tile_pool(name="ps_z", bufs=1,
                                                          space="PSUM"))
                    ps_bc = p23.enter_context(tc.tile_pool(name="ps_bc",
                                                           bufs=1,
                                                           space="PSUM"))
                    ps_o = p23.enter_context(tc.tile_pool(name="ps_o", bufs=2,
                                                          space="PSUM"))
                    ps_po = p23.enter_context(tc.tile_pool(name="ps_po",
                                                           bufs=2,
                                                           space="PSUM"))
                    bc_pool2 = p23.enter_context(tc.tile_pool(name="bc2",
                                                              bufs=2))
                    for h in range(HPC):
                        q_sb = qks[h]
                        bcq = bc_pool2.tile([128, 128, SB_N], f32, tag="bcq2")
                        nc.sync.dma_start(
                            bcq[:].rearrange("p a b -> p (a b)"),
                            IQD[h].rearrange("p b -> (p b)")[None, :]
                            .to_broadcast((128, 128 * SB_N)))
                        qv = q_sb[:].rearrange("p (blk sp) -> p sp blk",
                                               sp=128)
                        nc.vector.scalar_tensor_tensor(
                            qv, qv, gsc_sb[:], bcq[:], ALU.mult, ALU.mult)
                    wos = []
                    for mo in range(MO_N):
                        wo_sb = wo_pool.tile([128, HPC, 128], f32r,
                                             tag=f"wo{mo}", name=f"wo{mo}")
                        nc.scalar.dma_start(wo_sb[:], WOT[mo])
                        wos.append(wo_sb)
                    for c in range(SC_N):
                        for h in range(HPC):
                            q_sb, k_sb = qks[h], qks[HPC + h]
                            oc = ps_o.tile([128, 512], f32, tag="o")
                            zc = ps_z.tile([1, 512], f32, tag="z")
                            nblk = 4 * c + 4
                            for i in range(nblk):
                                r = i - 4 * c
                                off = 128 * r if r > 0 else 0
                                sps = ps_s.tile([128, 512], f32, tag="s")
                                nc.tensor.matmul(
                                    sps[:, off:512],
                                    k_sb[:, i * 128:(i + 1) * 128],
                                    q_sb[:, c * 512 + off:(c + 1) * 512],
                                    start=True, stop=True,
                                    skip_group_check=True)
                                pt = pt_pool.tile([128, 512], f32r, tag="pt")
                                nc.scalar.activation(
                                    pt[:, off:512], sps[:, off:512], AF.Exp,
                                    bias=mb_sb[:, i:i + 1],
                                    scale=inv_all[:, HPC + h, i:i + 1])
                                if r >= 0:
                                    nc.vector.tensor_tensor(
                                        pt[:, off:off + 128],
                                        pt[:, off:off + 128],
                                        tri_sb[:], ALU.mult)
                                nc.tensor.matmul(
                                    zc[:, off:512], ones_sb[:], pt[:, off:512],
                                    start=(i == 0), stop=(i == nblk - 1),
                                    skip_group_check=True)
                                nc.tensor.matmul(
                                    oc[:, off:512],
                                    v_sb[:, i, h * 128:(h + 1) * 128],
                                    pt[:, off:512],
                                    start=(i == 0), stop=(i == nblk - 1),
                                    skip_group_check=True)
                            nc.scalar.copy(
                                attn_sb[:, h, c * 512:(c + 1) * 512], oc[:])
                            # invZ via reciprocal + K=1 broadcast matmul
                            izrow = zs_pool.tile([1, 512], f32r, tag="izrow")
                            with nc.allow_low_precision(
                                    reason="invZ broadcast via f32r matmul"):
                                nc.vector.reciprocal(izrow[:], zc[:])
                            bcz_ps = ps_bc.tile([128, 512], f32, tag="bcz")
                            nc.tensor.matmul(bcz_ps[:], ones_row[:],
                                             izrow[:], start=True, stop=True,
                                             skip_group_check=True)
                            nc.vector.tensor_tensor(
                                attn_sb[:, h, c * 512:(c + 1) * 512],
                                attn_sb[:, h, c * 512:(c + 1) * 512],
                                bcz_ps[:], ALU.mult)
                        # output projection for this chunk
                        for mo in range(MO_N):
                            pp = ps_po.tile([128, 512], f32, tag="pp")
                            for jb in range(HPC):
                                nc.tensor.matmul(
                                    pp[:], wos[mo][:, jb, :],
                                    attn_sb[:, jb, c * 512:(c + 1) * 512],
                                    start=(jb == 0), stop=(jb == HPC - 1))
                            ob = po_pool.tile([128, 512], f32, tag="ob")
                            if mo % 2 == 0:
                                nc.scalar.copy(ob[:], pp[:])
                            else:
                                nc.vector.tensor_copy(ob[:], pp[:])
                            nc.gpsimd.dma_start(
                                PO[mo * 128:(mo + 1) * 128,
                                   c * 512:(c + 1) * 512], ob[:])
    nc.compile()
    return nc


def rope_tables(S):
    j = np.arange(0, HD, 2, dtype=np.float32)
    with np.errstate(over="ignore", divide="ignore"):
        freq = np.exp(np.float32(math.log(ROPE_BASE)) * j)
        inv_freq = (np.float32(1.0) / freq).astype(np.float32)
    t = np.arange(S, dtype=np.float32)
    freqs = t[:, None] * inv_freq[None, :]
    return np.cos(freqs).astype(np.float32), np.sin(freqs).astype(np.float32)


def pack_core(x2d, wq4, wk4, wv4, wo_sh, qnw, knw, mask_row, S, KD, MO):
    """Per-core input map. x2d: [S, KD]; wq4/wk4/wv4: [512, KD] weight rows for
    this core's heads; wo_sh: [MO, 512] = w_o columns for these heads."""
    SB_N = S // 128
    SC_N = S // 512
    MO_N = MO // 128
    KB = KD // 128
    f = np.float32
    xtc = np.ascontiguousarray(
        x2d.T.reshape(KB, 128, SC_N, 512).transpose(2, 1, 0, 3), dtype=f)
    wqk = np.empty((NHT, 128, KB, 128), f)
    for hh in range(HPC):
        wqk[hh] = wq4[hh * 128:(hh + 1) * 128, :].T.reshape(
            KB, 128, 128).transpose(1, 0, 2)
        wqk[HPC + hh] = wk4[hh * 128:(hh + 1) * 128, :].T.reshape(
            KB, 128, 128).transpose(1, 0, 2)
    wv = np.ascontiguousarray(
        wv4.T.reshape(KB, 128, HPC * 128).transpose(1, 0, 2), dtype=f)
    wot = np.ascontiguousarray(
        wo_sh.reshape(MO_N, 128, HPC, 128).transpose(0, 3, 2, 1), dtype=f)
    cos, sin = rope_tables(S)
    cc = np.empty((NR, S), f)
    ss = np.empty((NR, S), f)
    for ht in range(NHT):
        for jj in range(NROPE):
            cc[ht * NROPE + jj] = cos[:, jj]
            ss[ht * NROPE + jj] = sin[:, jj]
    gsc = (qnw * knw * np.float32(SCALE)).astype(f)
    mb = np.where(mask_row.reshape(SB_N, 128).T, f(0.0), f(NEG)).astype(f)
    tri = (np.arange(128)[None, :] >= np.arange(128)[:, None]).astype(f)
    return {
        "xtc": xtc, "wqk": wqk, "wv": wv, "wot": wot, "cc": cc, "ss": ss,
        "gsc": gsc, "mb": mb, "tri": tri, "one": np.ones(128, f),
    }


_cache = {}


def make_in_maps(inputs):
    x = np.asarray(inputs["x"], np.float32)
    w_q = np.asarray(inputs["w_q"], np.float32)
    w_k = np.asarray(inputs["w_k"], np.float32)
    w_v = np.asarray(inputs["w_v"], np.float32)
    w_o = np.asarray(inputs["w_o"], np.float32)
    qnw = np.asarray(inputs["q_norm_w"], np.float32)
    knw = np.asarray(inputs["k_norm_w"], np.float32)
    qmask = np.asarray(inputs["query_mask"]).astype(bool)
    in_maps = []
    for core in range(NCORES):
        b, hg = core // 4, core % 4
        rows = slice(hg * 512, (hg + 1) * 512)
        in_maps.append(pack_core(
            x[b], w_q[rows], w_k[rows], w_v[rows], w_o[:, rows],
            qnw, knw, qmask[b], S_FULL, HID, HID))
    return in_maps


def run_traced(inputs):
    if "nc" not in _cache:
        _cache["nc"] = build_kernel()
    return run_bass_kernel_spmd(_cache["nc"], make_in_maps(inputs),
                                core_ids=list(range(NCORES)), trace=True)


def kernel(**inputs):
    qmask = np.asarray(inputs["query_mask"]).astype(bool)
    if "nc" not in _cache:
        _cache["nc"] = build_kernel()
    nc = _cache["nc"]
    in_maps = make_in_maps(inputs)
    res = run_bass_kernel_spmd(nc, in_maps, core_ids=list(range(NCORES)))
    out = np.zeros((B, S_FULL, HID), np.float32)
    for core in range(NCORES):
        b = core // 4
        out[b] += res.results[core]["po"].T
    out = np.where(qmask[:, :, None], out, np.float32(0.0))
    return out.astype(np.float32)


# revision 44
# speedup vs baseline: 1.1260x; 1.0846x over previous
"""Trainium2 Bass kernel for nn_AttentionBlock (B=2, S=2048, HID=2048, 16 heads, HD=128).

Sharding: 8 cores = 2 (batch) x 4 (head-groups of 4 heads). Each core computes
q/k/v projections for its heads, RoPE + per-head QK RMSNorm, causal attention,
and a partial w_o projection; the host sums the 4 partial outputs per batch.

All matmuls run as float32r (single-pass, ~fp22 operand precision, fp32
accumulate). Activations stay fp32 on-chip. x is streamed in s-column chunks
so q/k stay SBUF-resident end-to-end (no DRAM spill).
"""
import sys
import math
from contextlib import ExitStack

sys.path.insert(0, "/opt/trn_rl_repo")

import numpy as np

import concourse.bass as bass
import concourse.tile as tile
from concourse import bacc, mybir
from concourse.bass_utils import run_bass_kernel_spmd

f32 = mybir.dt.float32
f32r = mybir.dt.float32r
AF = mybir.ActivationFunctionType
ALU = mybir.AluOpType

B, S_FULL, HID = 2, 2048, 2048
NH, HD = 16, 128
HPC = 4                      # heads per core
NCORES = 8
ROPE_BASE = 10000.0
EPS = 1e-6
SCALE = 1.0 / math.sqrt(HD)
NEG = -1e30
NROPE = 3                    # rope pairs actually rotated (j>=3: |sin| < 2.1e-9)
NHT = 2 * HPC                # 8 head-tensors per core (4 q + 4 k)
NR = NHT * NROPE             # packed rope rows


def build_kernel(S=S_FULL, KD=HID, MO=HID, R=1):
    KB = KD // 128
    SC_N = S // 512
    SB_N = S // 128
    MO_N = MO // 128

    nc = bacc.Bacc("TRN2", target_bir_lowering=False, debug=False,
                   num_devices=NCORES)
    XTC = nc.dram_tensor("xtc", [SC_N, 128, KB, 512], f32r,
                         kind="ExternalInput").ap()
    WQK = nc.dram_tensor("wqk", [NHT, 128, KB, 128], f32r,
                         kind="ExternalInput").ap()
    WV = nc.dram_tensor("wv", [128, KB, HPC * 128], f32r,
                        kind="ExternalInput").ap()
    WOT = nc.dram_tensor("wot", [MO // 128, 128, HPC, 128], f32r,
                         kind="ExternalInput").ap()
    CC = nc.dram_tensor("cc", [NR, S], f32, kind="ExternalInput").ap()
    SS = nc.dram_tensor("ss", [NR, S], f32, kind="ExternalInput").ap()
    GSC = nc.dram_tensor("gsc", [128], f32, kind="ExternalInput").ap()
    MB = nc.dram_tensor("mb", [128, SB_N], f32, kind="ExternalInput").ap()
    TRI = nc.dram_tensor("tri", [128, 128], f32r, kind="ExternalInput").ap()
    ONE = nc.dram_tensor("one", [128], f32r, kind="ExternalInput").ap()
    PO = nc.dram_tensor("po", [MO, S], f32, kind="ExternalOutput").ap()
    # internal DRAM scratch (invq partition-broadcast bounce)
    IQD = nc.dram_tensor("invq_d", [HPC, 128, SB_N], f32).ap()

    with tile.TileContext(nc) as tc, ExitStack() as gctx:
        singles = gctx.enter_context(tc.tile_pool(name="singles", bufs=1))
        gsc_sb = singles.tile([128, 1], f32)
        nc.sync.dma_start(gsc_sb[:], GSC[:, None])
        mb_sb = singles.tile([128, SB_N], f32)
        nc.gpsimd.dma_start(mb_sb[:], MB)
        tri_sb = singles.tile([128, 128], f32r)
        nc.gpsimd.dma_start(tri_sb[:], TRI)
        ones_sb = singles.tile([128, 1], f32r)
        nc.gpsimd.dma_start(ones_sb[:], ONE[:, None])
        ones_row = singles.tile([1, 128], f32r)
        nc.gpsimd.dma_start(ones_row[:], ONE[None, :])
        ones2_sb = singles.tile([128, 2], f32r)
        nc.sync.dma_start(ones2_sb[:, 0:1], ONE[:, None])
        nc.sync.dma_start(ones2_sb[:, 1:2], ONE[:, None])
        eps_sb = singles.tile([128, 1], f32)
        nc.vector.memset(eps_sb[:], EPS)

        for _rep in range(R):
            with ExitStack() as rctx:
                rs = rctx.enter_context(tc.tile_pool(name="res", bufs=1))
                inv_all = rs.tile([128, NHT, SB_N], f32, name="inv_all")
                srt = rs.tile([128, NHT, SB_N], f32, name="srt")
                v_sb = rs.tile([128, SB_N, HPC * 128], f32r, name="v_sb")
                qks = [rs.tile([128, S], f32r, name=f"qk{t}")
                       for t in range(NHT)]
                # ---- P1: projections, x streamed in s-column chunks ----
                with ExitStack() as p1:
                    wv_pool = p1.enter_context(tc.tile_pool(name="wvp",
                                                            bufs=1))
                    wv_sb = wv_pool.tile([128, KB, HPC * 128], f32r,
                                         name="wv_sb")
                    for kb in range(0, KB, 2):
                        nc.gpsimd.dma_start(wv_sb[:, kb, :], WV[:, kb, :])
                    xt_pool = p1.enter_context(tc.tile_pool(name="xtc", bufs=5))
                    w_pool = p1.enter_context(tc.tile_pool(name="wqk", bufs=2))
                    sq_pool = p1.enter_context(tc.tile_pool(name="sq", bufs=2))
                    rp_pool = p1.enter_context(tc.tile_pool(name="rope",
                                                             bufs=1))
                    ps_pool = p1.enter_context(
                        tc.tile_pool(name="ps1", bufs=5, space="PSUM"))
                    ssq_pool = p1.enter_context(
                        tc.tile_pool(name="ssq", bufs=1, space="PSUM"))
                    ssq_ps = ssq_pool.tile([128, NHT, SB_N, 2], f32)
                    KQ = KB // 4
                    for c in range(SC_N):
                        xtq = []
                        for qq in range(4):
                            xq = xt_pool.tile([128, KQ, 512], f32r, tag="xtc",
                                              name=f"xt{qq}")
                            nc.sync.dma_start(
                                xq[:], XTC[c, :, qq * KQ:(qq + 1) * KQ, :])
                            xtq.append(xq)
                        if c == 0:
                            for kb in range(1, KB, 2):
                                nc.sync.dma_start(wv_sb[:, kb, :],
                                                  WV[:, kb, :])

                        def xts(kb):
                            return xtq[kb // KQ][:, kb % KQ, :]
                        for ht in range(NHT):
                            w_sb = w_pool.tile([128, KB, 128], f32r, tag="w")
                            if c == 0 and ht == 0:
                                for qq in range(4):
                                    nc.scalar.dma_start(
                                        w_sb[:, qq * KQ:(qq + 1) * KQ, :],
                                        WQK[ht, :, qq * KQ:(qq + 1) * KQ, :])
                            else:
                                nc.scalar.dma_start(w_sb[:], WQK[ht])
                            psq = ps_pool.tile([128, 512], f32, tag="ps1t")
                            for kb in range(KB):
                                nc.tensor.matmul(
                                    psq[:], w_sb[:, kb, :], xts(kb),
                                    start=(kb == 0), stop=(kb == KB - 1))
                            nc.vector.tensor_copy(
                                qks[ht][:, c * 512:(c + 1) * 512], psq[:])
                            sq = sq_pool.tile([128, 512], f32r, tag="sqt")
                            nc.scalar.activation(sq[:], psq[:], AF.Square)
                            for s2 in range(4):
                                blk = c * 4 + s2
                                nc.tensor.matmul(
                                    ssq_ps[:, ht, blk, :],
                                    sq[:, s2 * 128:(s2 + 1) * 128],
                                    ones2_sb[:], start=True, stop=True,
                                    skip_group_check=True)
                        if c == SC_N - 1:
                            # invrms = 1/sqrt(ssq/HD + eps); emitted before
                            # rope so recip isn't queued behind it on DVE
                            nc.scalar.activation(
                                srt[:].rearrange("p a b -> p (a b)"),
                                ssq_ps[:, :, :, 0].rearrange(
                                    "p a b -> p (a b)"),
                                AF.Sqrt, bias=eps_sb[:], scale=1.0 / HD)
                            nc.vector.reciprocal(
                                inv_all[:].rearrange("p a b -> p (a b)"),
                                srt[:].rearrange("p a b -> p (a b)"))
                            for h in range(HPC):
                                nc.sync.dma_start(IQD[h], inv_all[:, h, :])
                        # per-chunk packed rope on the 6 active rows/ht
                        cs = slice(c * 512, (c + 1) * 512)
                        X1 = rp_pool.tile([NR, 512], f32, tag="X1")
                        X2 = rp_pool.tile([NR, 512], f32, tag="X2")
                        for ht in range(NHT):
                            nc.scalar.dma_start(
                                X1[ht * NROPE:(ht + 1) * NROPE, :],
                                qks[ht][0:NROPE, cs].bitcast(f32))
                            nc.scalar.dma_start(
                                X2[ht * NROPE:(ht + 1) * NROPE, :],
                                qks[ht][64:64 + NROPE, cs].bitcast(f32))
                        cc_sb = rp_pool.tile([NR, 512], f32, tag="ccs")
                        ss_sb = rp_pool.tile([NR, 512], f32, tag="sss")
                        nc.sync.dma_start(cc_sb[:], CC[:, cs])
                        nc.sync.dma_start(ss_sb[:], SS[:, cs])
                        tA = rp_pool.tile([NR, 512], f32, tag="tA")
                        tB = rp_pool.tile([NR, 512], f32, tag="tB")
                        nc.vector.tensor_tensor(tA[:], X1[:], cc_sb[:],
                                                ALU.mult)
                        nc.vector.tensor_tensor(X1[:], X1[:], ss_sb[:],
                                                ALU.mult)
                        nc.vector.tensor_tensor(tB[:], X2[:], cc_sb[:],
                                                ALU.mult)
                        nc.vector.tensor_tensor(X2[:], X2[:], ss_sb[:],
                                                ALU.mult)
                        nc.vector.tensor_tensor(tA[:], tA[:], X2[:],
                                                ALU.subtract)
                        nc.vector.tensor_tensor(tB[:], tB[:], X1[:], ALU.add)
                        for ht in (0, 4, 1, 5, 2, 6, 3, 7):
                            nc.gpsimd.dma_start(
                                qks[ht][0:NROPE, cs],
                                tA[ht * NROPE:(ht + 1) * NROPE, :]
                                .bitcast(f32r))
                            nc.gpsimd.dma_start(
                                qks[ht][64:64 + NROPE, cs],
                                tB[ht * NROPE:(ht + 1) * NROPE, :]
                                .bitcast(f32r))
                        # V projection for this chunk's 4 s-blocks
                        for s2 in range(4):
                            sb = c * 4 + s2
                            psv = ps_pool.tile([128, HPC * 128], f32,
                                               tag="ps1t")
                            for kb in range(KB):
                                nc.tensor.matmul(
                                    psv[:],
                                    xts(kb)[:, s2 * 128:(s2 + 1) * 128],
                                    wv_sb[:, kb, :],
                                    start=(kb == 0), stop=(kb == KB - 1))
                            nc.scalar.copy(v_sb[:, sb, :], psv[:])

                # ---- P2/P3: attention + output proj, chunk-outer ----
                with ExitStack() as p23:
                    at_pool = p23.enter_context(tc.tile_pool(name="attn",
                                                             bufs=1))
                    attn_sb = at_pool.tile([128, HPC, S], f32r)
                    wo_pool = p23.enter_context(tc.tile_pool(name="wo",
                                                             bufs=1))
                    pt_pool = p23.enter_context(tc.tile_pool(name="pt",
                                                             bufs=4))
                    zs_pool = p23.enter_context(tc.tile_pool(name="zs",
                                                             bufs=2))
                    po_pool = p23.enter_context(tc.tile_pool(name="po",
                                                             bufs=3))
                    ps_s = p23.enter_context(tc.tile_pool(name="ps_s", bufs=2,
                                                          space="PSUM"))
                    ps_z = p23.enter_context(tc.tile_pool(name="ps_z", bufs=1,
                                                          space="PSUM"))
                    ps_bc = p23.enter_context(tc.tile_pool(name="ps_bc",
                                                           bufs=1,
                                                           space="PSUM"))
                    ps_o = p23.enter_context(tc.tile_pool(name="ps_o", bufs=2,
                                                          space="PSUM"))
                    ps_po = p23.enter_context(tc.tile_pool(name="ps_po",
                                                           bufs=2,
                                                           space="PSUM"))
                    bc_pool2 = p23.enter_context(tc.tile_pool(name="bc2",
                                                              bufs=2))
                    for h in range(HPC):
                        q_sb = qks[h]
                        bcq = bc_pool2.tile([128, 128, SB_N], f32, tag="bcq2")
                        nc.sync.dma_start(
                            bcq[:].rearrange("p a b -> p (a b)"),
                            IQD[h].rearrange("p b -> (p b)")[None, :]
                            .to_broadcast((128, 128 * SB_N)))
                        qv = q_sb[:].rearrange("p (blk sp) -> p sp blk",
                                               sp=128)
                        nc.vector.scalar_tensor_tensor(
                            qv, qv, gsc_sb[:], bcq[:], ALU.mult, ALU.mult)
                    wos = []
                    for mo in range(MO_N):
                        wo_sb = wo_pool.tile([128, HPC, 128], f32r,
                                             tag=f"wo{mo}", name=f"wo{mo}")
                        nc.scalar.dma_start(wo_sb[:], WOT[mo])
                        wos.append(wo_sb)
                    for c in range(SC_N):
                        for h in range(HPC):
                            q_sb, k_sb = qks[h], qks[HPC + h]
                            oc = ps_o.tile([128, 512], f32, tag="o")
                            zc = ps_z.tile([1, 512], f32, tag="z")
                            nblk = 4 * c + 4
                            for i in range(nblk):
                                r = i - 4 * c
                                off = 128 * r if r > 0 else 0
                                sps = ps_s.tile([128, 512], f32, tag="s")
                                nc.tensor.matmul(
                                    sps[:, off:512],
                                    k_sb[:, i * 128:(i + 1) * 128],
                                    q_sb[:, c * 512 + off:(c + 1) * 512],
                                    start=True, stop=True,
                                    skip_group_check=True)
                                pt = pt_pool.tile([128, 512], f32r, tag="pt")
                                nc.scalar.activation(
                                    pt[:, off:512], sps[:, off:512], AF.Exp,
                                    bias=mb_sb[:, i:i + 1],
                                    scale=inv_all[:, HPC + h, i:i + 1])
                                if r >= 0:
                                    nc.vector.tensor_tensor(
                                        pt[:, off:off + 128],
                                        pt[:, off:off + 128],
                                        tri_sb[:], ALU.mult)
                                nc.tensor.matmul(
                                    zc[:, off:512], ones_sb[:], pt[:, off:512],
                                    start=(i == 0), stop=(i == nblk - 1),
                                    skip_group_check=True)
                                nc.tensor.matmul(
                                    oc[:, off:512],
                                    v_sb[:, i, h * 128:(h + 1) * 128],
                                    pt[:, off:512],
                                    start=(i == 0), stop=(i == nblk - 1),
                                    skip_group_check=True)
                            nc.scalar.copy(
                                attn_sb[:, h, c * 512:(c + 1) * 512], oc[:])
                            # invZ via reciprocal + K=1 broadcast matmul
                            izrow = zs_pool.tile([1, 512], f32r, tag="izrow")
                            with nc.allow_low_precision(
                                    reason="invZ broadcast via f32r matmul"):
                                nc.vector.reciprocal(izrow[:], zc[:])
                            bcz_ps = ps_bc.tile([128, 512], f32, tag="bcz")
                            nc.tensor.matmul(bcz_ps[:], ones_row[:],
                                             izrow[:], start=True, stop=True,
                                             skip_group_check=True)
                            nc.vector.tensor_tensor(
                                attn_sb[:, h, c * 512:(c + 1) * 512],
                                attn_sb[:, h, c * 512:(c + 1) * 512],
                                bcz_ps[:], ALU.mult)
                        # output projection for this chunk
                        for mo in range(MO_N):
                            pp = ps_po.tile([128, 512], f32, tag="pp")
                            for jb in range(HPC):
                                nc.tensor.matmul(
                                    pp[:], wos[mo][:, jb, :],
                                    attn_sb[:, jb, c * 512:(c + 1) * 512],
                                    start=(jb == 0), stop=(jb == HPC - 1))
                            ob = po_pool.tile([128, 512], f32, tag="ob")
                            if mo % 2 == 0:
                                nc.scalar.copy(ob[:], pp[:])
                            else:
                                nc.vector.tensor_copy(ob[:], pp[:])
                            if c == SC_N - 1:
                                poeng = nc.sync if mo % 2 == 0 else nc.scalar
                            else:
                                poeng = nc.gpsimd
                            poeng.dma_start(
                                PO[mo * 128:(mo + 1) * 128,
                                   c * 512:(c + 1) * 512], ob[:])
    nc.compile()
    return nc


def rope_tables(S):
    j = np.arange(0, HD, 2, dtype=np.float32)
    with np.errstate(over="ignore", divide="ignore"):
        freq = np.exp(np.float32(math.log(ROPE_BASE)) * j)
        inv_freq = (np.float32(1.0) / freq).astype(np.float32)
    t = np.arange(S, dtype=np.float32)
    freqs = t[:, None] * inv_freq[None, :]
    return np.cos(freqs).astype(np.float32), np.sin(freqs).astype(np.float32)


def pack_core(x2d, wq4, wk4, wv4, wo_sh, qnw, knw, mask_row, S, KD, MO):
    """Per-core input map. x2d: [S, KD]; wq4/wk4/wv4: [512, KD] weight rows for
    this core's heads; wo_sh: [MO, 512] = w_o columns for these heads."""
    SB_N = S // 128
    SC_N = S // 512
    MO_N = MO // 128
    KB = KD // 128
    f = np.float32
    xtc = np.ascontiguousarray(
        x2d.T.reshape(KB, 128, SC_N, 512).transpose(2, 1, 0, 3), dtype=f)
    wqk = np.empty((NHT, 128, KB, 128), f)
    for hh in range(HPC):
        wqk[hh] = wq4[hh * 128:(hh + 1) * 128, :].T.reshape(
            KB, 128, 128).transpose(1, 0, 2)
        wqk[HPC + hh] = wk4[hh * 128:(hh + 1) * 128, :].T.reshape(
            KB, 128, 128).transpose(1, 0, 2)
    wv = np.ascontiguousarray(
        wv4.T.reshape(KB, 128, HPC * 128).transpose(1, 0, 2), dtype=f)
    wot = np.ascontiguousarray(
        wo_sh.reshape(MO_N, 128, HPC, 128).transpose(0, 3, 2, 1), dtype=f)
    cos, sin = rope_tables(S)
    cc = np.empty((NR, S), f)
    ss = np.empty((NR, S), f)
    for ht in range(NHT):
        for jj in range(NROPE):
            cc[ht * NROPE + jj] = cos[:, jj]
            ss[ht * NROPE + jj] = sin[:, jj]
    gsc = (qnw * knw * np.float32(SCALE)).astype(f)
    mb = np.where(mask_row.reshape(SB_N, 128).T, f(0.0), f(NEG)).astype(f)
    tri = (np.arange(128)[None, :] >= np.arange(128)[:, None]).astype(f)
    return {
        "xtc": xtc, "wqk": wqk, "wv": wv, "wot": wot, "cc": cc, "ss": ss,
        "gsc": gsc, "mb": mb, "tri": tri, "one": np.ones(128, f),
    }


_cache = {}


def make_in_maps(inputs):
    x = np.asarray(inputs["x"], np.float32)
    w_q = np.asarray(inputs["w_q"], np.float32)
    w_k = np.asarray(inputs["w_k"], np.float32)
    w_v = np.asarray(inputs["w_v"], np.float32)
    w_o = np.asarray(inputs["w_o"], np.float32)
    qnw = np.asarray(inputs["q_norm_w"], np.float32)
    knw = np.asarray(inputs["k_norm_w"], np.float32)
    qmask = np.asarray(inputs["query_mask"]).astype(bool)
    in_maps = []
    for core in range(NCORES):
        b, hg = core // 4, core % 4
        rows = slice(hg * 512, (hg + 1) * 512)
        in_maps.append(pack_core(
            x[b], w_q[rows], w_k[rows], w_v[rows], w_o[:, rows],
            qnw, knw, qmask[b], S_FULL, HID, HID))
    return in_maps


def run_traced(inputs):
    if "nc" not in _cache:
        _cache["nc"] = build_kernel()
    return run_bass_kernel_spmd(_cache["nc"], make_in_maps(inputs),
                                core_ids=list(range(NCORES)), trace=True)


def kernel(**inputs):
    qmask = np.asarray(inputs["query_mask"]).astype(bool)
    if "nc" not in _cache:
        _cache["nc"] = build_kernel()
    nc = _cache["nc"]
    in_maps = make_in_maps(inputs)
    res = run_bass_kernel_spmd(nc, in_maps, core_ids=list(range(NCORES)))
    out = np.zeros((B, S_FULL, HID), np.float32)
    for core in range(NCORES):
        b = core // 4
        out[b] += res.results[core]["po"].T
    out = np.where(qmask[:, :, None], out, np.float32(0.0))
    return out.astype(np.float32)
